# revision 19
# baseline (speedup 1.0000x reference)
"""Brute-force KNN density estimator on 8 Trainium2 NeuronCores.

reference math:
    dist[i, j] = ||x_i - x_j||_2 over features [8192, 1024]
    kth[i] = 6th smallest of dist[i, :]  (self-distance included)
    out[i] = 1 / (kth[i] + 1e-8)

Strategy (data-parallel over query rows, 1024 rows per core):
    - Rank rows of the distance matrix by T[i,j] = 2*G[i,j] - (sq[j] - mean(sq))
      (per-row-constant sq[i] and the monotone sqrt don't change ranking).
    - fp8 e4m3 DoubleRow matmuls accumulate 2*G; the -(sq[j]-sbar) bias comes
      from a ScalarE PSUM pre-seed (start=False groups) for t>=1, and from a
      DVE tensor_sub after a plain start=True group for t==0 — the t==0
      start=True groups also normalize every PSUM bank's pending-zero state,
      so no bank-clearing warmup matmuls are needed.
    - Seeds for column-tile t+1 are emitted at the END of tile t's groups
      (pre-allocated PSUM tiles), giving ScalarE a full bank-cycle (~6.8us)
      of slack instead of racing the PE group starts just-in-time.
    - VectorE: single MAX8 per [128, 512] PSUM tile -> per-tile top-8
      candidates; a half-merge at t=7 plus a 72-element final at t=15 keep
      the end-of-run DVE drain short. The kernel outputs raw top-8 T values;
      the host recovers kth_d2 = (sq[i] + sbar) - T6 with exact fp32 norms.
    - Head engineering: HBM flow starts ~8.4us and transfers round-robin, so
      DMAs issue in strict need order split over the two HWDGE rings (SP:
      ft/sqc + late qt tiles; ACT: early qt tiles).  A dummy ACTIVATE pulls
      the 1.3us ACT_TABLE_LOAD into the head; ~22 tiny warm matmuls keep the
      PE busy from engine-ready (~7.1us) until real operands land (~9.4us)
      so the HAM clock gate ramps to 2.4 GHz as early as possible.
"""

import os

import numpy as np
import ml_dtypes

N = 8192          # points
D = 1024          # feature dim
NCORES = 8
ROWS = N // NCORES   # rows (queries) per core
RT = ROWS // 128     # row tiles per core
CTILE = 512          # matmul moving free dim
CT = N // CTILE      # column tiles
KC = D // 128        # 128-row contraction chunks
K_ORD = 5            # 0-based rank -> 6th smallest
EPS = 1e-8
WARM_MM = 24         # tiny N=128 warm matmuls (~2.5us cold) for the HAM ramp

TRACE = bool(int(os.environ.get("KNN_TRACE", "0")))
LAST_EXEC_NS = None

def _build_nc():
    import concourse.mybir as mybir
    from concourse import bacc
    from concourse.tile import TileContext

    dt = mybir.dt
    nc = bacc.Bacc(None, target_bir_lowering=False, enable_partition_id=False)

    # per-tile layout [CT][128 part][KC*CTILE contiguous] -> one DMA per tile
    ft_d = nc.dram_tensor("ft", [CT, 128, KC * CTILE], dt.float8e4, kind="ExternalInput")
    # query cols r-major per partition: the r0/r1 pair is one 2KB-line DMA
    # (what the first two row groups gate on) and r2..r7 one 6KB-line DMA —
    # wide per-partition lines roughly double the descriptor-limited DMA rate
    # vs eight 1KB-line transfers
    qt_d = nc.dram_tensor("qt", [128, RT * KC * 128], dt.float8e4, kind="ExternalInput")
    sqc_d = nc.dram_tensor("sqc", [128, N], dt.bfloat16, kind="ExternalInput")
    # raw per-row-tile top-8 T values; the density epilogue runs on the host
    out_d = nc.dram_tensor("out", [128, RT * 8], dt.float32, kind="ExternalOutput")

    DR = mybir.MatmulPerfMode.DoubleRow
    Copy = mybir.ActivationFunctionType.Copy

    with TileContext(nc) as tc:
        with (
            tc.tile_pool(name="persist", bufs=1) as persist,
            tc.tile_pool(name="ftp", bufs=3) as ftp,
            tc.tile_pool(name="psum", bufs=8, space="PSUM") as psum,
        ):
            # r-tile-major layout: each query r-tile is a fully contiguous
            # [128, KC*128] block -> its DMA is one descriptor per partition
            qt_s = persist.tile([128, RT, KC, 128], dt.float8e4)
            sqc_s = persist.tile([128, N], dt.bfloat16)
            # per r: 16 tile top-8 slots + one half-merge slot at offset 128
            cand = persist.tile([128, RT, CT * 8 + 8], dt.float32)
            top8s = persist.tile([128, RT, 8], dt.float32)
            warm = persist.tile([128, 128], dt.bfloat16)
            act_scr = persist.tile([128, 8], dt.float32)

            def _qt_src(r0, r1):
                return qt_d[:, r0 * KC * 128:r1 * KC * 128].rearrange(
                    "p (r k i) -> p r k i", r=r1 - r0, k=KC)

            def _ft_src(t):
                return ft_d[t].rearrange("p (k j) -> p k j", k=KC)

            # ---- head DMA issue: need order over two HWDGE rings.  Early
            # transfers round-robin the HBM pipe, so only what the first row
            # groups gate on (qt tiles, ft0, ft1) may be in flight early;
            # sqc chunks issue behind ft2 in the loop.
            # ACT ring: qt r0/r1 pair, table-load trigger, qt r2..r7.
            nc.gpsimd.memset(warm, 0.0)
            nc.scalar.dma_start(qt_s[:, 0:2], _qt_src(0, 2))
            nc.scalar.activation(act_scr, warm[:, 0:8], Copy)
            nc.scalar.dma_start(qt_s[:, 2:RT], _qt_src(2, RT))
            # SP ring: ft0 in two k-halves, then ft1.
            ft_t0 = ftp.tile([128, KC, CTILE], dt.float8e4, tag="ft")
            nc.sync.dma_start(ft_t0[:, 0:KC // 2], _ft_src(0)[:, 0:KC // 2])
            nc.sync.dma_start(ft_t0[:, KC // 2:], _ft_src(0)[:, KC // 2:])
            ft_tiles = [ft_t0]
            ft_t1 = ftp.tile([128, KC, CTILE], dt.float8e4, tag="ft")
            nc.sync.dma_start(ft_t1, _ft_src(1))
            ft_tiles.append(ft_t1)
            nc.sync.dma_start(sqc_s[:, 0:1024], sqc_d[:, 0:1024])

            # warm-up: PE busy from engine-ready until real operands land.
            # No PSUM state is consumed downstream (t=0 groups use start=True).
            wps = [psum.tile([128, CTILE], dt.float32, tag="ps",
                             name=f"wps{b}") for b in range(3)]
            for i in range(WARM_MM):
                nc.tensor.matmul(wps[i % 3][:, 0:128], lhsT=warm, rhs=warm,
                                 start=True, stop=True)

            out_v = out_d.rearrange("p (r e) -> p r e", r=RT)
            ps_cur = [None] * RT
            for t in range(CT):
                if t == 1:
                    # remaining sqc chunks issue here so their transfers stay
                    # out of the congested head window
                    nc.sync.dma_start(sqc_s[:, 1024:4096], sqc_d[:, 1024:4096])
                if t < 2:
                    ft_t = ft_tiles[t]
                else:
                    if t == 2:
                        nc.sync.dma_start(sqc_s[:, 4096:N], sqc_d[:, 4096:N])
                    ft_t = ftp.tile([128, KC, CTILE], dt.float8e4, tag="ft")
                    nc.sync.dma_start(ft_t, _ft_src(t))
                sqc_t = sqc_s[:, t * CTILE:(t + 1) * CTILE]
                for r in range(RT):
                    if t == 0:
                        # start=True group: resets the bank's pending-zero
                        # state (left by the previous NEFF) and needs no
                        # pre-seed; DVE applies the -(sq[j]-sbar) bias after
                        ps = psum.tile([128, CTILE], dt.float32, tag="ps")
                        for k in range(0, KC, 2):
                            nc.tensor.matmul(
                                ps,
                                lhsT=qt_s[:, r, k:k + 2, :],
                                rhs=ft_t[:, k:k + 2, :],
                                start=(k == 0),
                                stop=(k == KC - 2),
                                perf_mode=DR,
                                skip_group_check=True,
                            )
                        nc.vector.tensor_sub(ps, ps, sqc_t)
                    else:
                        # bank was pre-seeded with -(sq[j]-sbar) at the end
                        # of tile t-1; fp8 matmuls accumulate 2*G on top
                        ps = ps_cur[r]
                        for k in range(0, KC, 2):
                            nc.tensor.matmul(
                                ps,
                                lhsT=qt_s[:, r, k:k + 2, :],
                                rhs=ft_t[:, k:k + 2, :],
                                start=False,
                                stop=(k == KC - 2),
                                perf_mode=DR,
                                skip_group_check=True,
                            )
                    nc.vector.max(
                        out=cand[:, r, t * 8:(t + 1) * 8],
                        in_=ps,
                    )
                    if t == CT // 2 - 1:
                        # half-merge the first 8 tiles' candidates into slot
                        # 128:136 while DVE has slack, so the t=15 final scans
                        # 72 elements instead of 128
                        nc.vector.max(out=cand[:, r, CT * 8:CT * 8 + 8],
                                      in_=cand[:, r, 0:CT * 4])
                    if t == CT - 1:
                        # final top-8 for row-tile r: second-half slots + the
                        # half-merge slot, issued as soon as r's last tile is
                        # done so the tail after the last matmul stays short
                        nc.vector.max(out=top8s[:, r, :],
                                      in_=cand[:, r, CT * 4:CT * 8 + 8])
                        if r == RT - 2:
                            # overlap most of the output DMA with r=7's tail
                            nc.sync.dma_start(out_v[:, 0:RT - 1], top8s[:, 0:RT - 1])
                # pre-allocate + seed tile t+1's PSUM banks now: each seed is
                # gated only on its bank's t-tile readers (sub/max8), so
                # ScalarE runs a full bank cycle ahead of the PE group starts
                if t + 1 < CT:
                    sqc_n = sqc_s[:, (t + 1) * CTILE:(t + 2) * CTILE]
                    nxt = []
                    for r in range(RT):
                        p = psum.tile([128, CTILE], dt.float32, tag="ps")
                        nc.scalar.activation(p, sqc_n, Copy, scale=-1.0)
                        nxt.append(p)
                    ps_cur = nxt

            nc.sync.dma_start(out_v[:, RT - 1:RT], top8s[:, RT - 1:RT])

    # run Bacc's passes (register allocation, event-semaphore wait splitting)
    # before handing off to the PJRT path, which binds without finalizing
    nc.finalize()
    return nc


def kernel(features):
    global LAST_EXEC_NS
    from concourse.bass_utils import run_bass_kernel_spmd

    f32 = np.ascontiguousarray(np.asarray(features, dtype=np.float32))
    assert f32.shape == (N, D)

    sq = np.einsum("nd,nd->n", f32, f32, dtype=np.float32)   # exact fp32 norms
    sbar = float(sq.mean())
    ftq = f32.T.astype(ml_dtypes.float8_e4m3fn)               # [D, N] fp8
    # moving operand pre-scaled by 2 (exact in fp8) so PSUM accumulates 2*G
    ft2 = (ftq.astype(np.float32) * 2.0).astype(ml_dtypes.float8_e4m3fn)
    # [D, N] -> [CT, 128, KC*CTILE]: per column tile, partition p holds all
    # KC chunks contiguously -> a single fully-contiguous DMA per tile
    ft_tiles = np.ascontiguousarray(
        ft2.reshape(KC, 128, CT, CTILE).transpose(2, 1, 0, 3).reshape(CT, 128, KC * CTILE)
    )
    sqc_rep = np.ascontiguousarray(
        np.broadcast_to((sq - sbar).astype(ml_dtypes.bfloat16), (128, N))
    )

    in_maps = []
    for c in range(NCORES):
        lo = c * ROWS
        # [128, RT*KC*128]: partition p holds all r-tiles' k-chunks r-major,
        # so qt ships as two wide-line DMAs (r0/r1, r2..r7)
        qt = np.ascontiguousarray(
            ftq[:, lo:lo + ROWS].reshape(KC, 128, RT, 128)
            .transpose(1, 2, 0, 3).reshape(128, RT * KC * 128)
        )
        in_maps.append({"ft": ft_tiles, "qt": qt, "sqc": sqc_rep})

    nc = _build_nc()
    res = run_bass_kernel_spmd(nc, in_maps, core_ids=list(range(NCORES)), trace=TRACE)
    LAST_EXEC_NS = res.exec_time_ns

    # host epilogue with exact fp32 norms: T6[p, r] holds the 6th-largest
    # 2G-sqc for global row c*1024 + r*128 + p; kth_d2 = sq[i] + sbar - T6
    dens = []
    for c in range(NCORES):
        t6 = res.results[c]["out"].reshape(128, RT, 8)[:, :, K_ORD]   # [128, RT]
        sqi = (sq[c * ROWS:(c + 1) * ROWS] + sbar).reshape(RT, 128).T
        kd = np.maximum(sqi.astype(np.float32) - t6, 0.0, dtype=np.float32)
        dens.append((1.0 / (np.sqrt(kd) + EPS)).T.reshape(-1))
    return np.concatenate(dens).astype(np.float32)[:, None]


# revision 21
# speedup vs baseline: 1.0131x; 1.0131x over previous
"""Brute-force KNN density estimator on 8 Trainium2 NeuronCores.

reference math:
    dist[i, j] = ||x_i - x_j||_2 over features [8192, 1024]
    kth[i] = 6th smallest of dist[i, :]  (self-distance included)
    out[i] = 1 / (kth[i] + 1e-8)

Strategy (data-parallel over query rows, 1024 rows per core):
    - Rank rows of the distance matrix by T[i,j] = 2*G[i,j] - (sq[j] - mean(sq))
      (per-row-constant sq[i] and the monotone sqrt don't change ranking).
    - fp8 e4m3 DoubleRow matmuls accumulate 2*G; the -(sq[j]-sbar) bias comes
      from a ScalarE PSUM pre-seed (start=False groups) for t>=1, and from a
      DVE tensor_sub after a plain start=True group for t==0 — the t==0
      start=True groups also normalize every PSUM bank's pending-zero state,
      so no bank-clearing warmup matmuls are needed.
    - Seeds for column-tile t+1 are emitted at the END of tile t's groups
      (pre-allocated PSUM tiles), giving ScalarE a full bank-cycle (~6.8us)
      of slack instead of racing the PE group starts just-in-time.
    - VectorE: single MAX8 per [128, 512] PSUM tile -> per-tile top-8
      candidates; a half-merge at t=7 plus a 72-element final at t=15 keep
      the end-of-run DVE drain short. The kernel outputs raw top-8 T values;
      the host recovers kth_d2 = (sq[i] + sbar) - T6 with exact fp32 norms.
    - Head engineering: HBM flow starts ~8.4us and transfers round-robin, so
      DMAs issue in strict need order split over the two HWDGE rings (SP:
      ft/sqc + late qt tiles; ACT: early qt tiles).  A dummy ACTIVATE pulls
      the 1.3us ACT_TABLE_LOAD into the head; ~22 tiny warm matmuls keep the
      PE busy from engine-ready (~7.1us) until real operands land (~9.4us)
      so the HAM clock gate ramps to 2.4 GHz as early as possible.
"""

import os

import numpy as np
import ml_dtypes

N = 8192          # points
D = 1024          # feature dim
NCORES = 8
ROWS = N // NCORES   # rows (queries) per core
RT = ROWS // 128     # row tiles per core
CTILE = 512          # matmul moving free dim
CT = N // CTILE      # column tiles
KC = D // 128        # 128-row contraction chunks
K_ORD = 5            # 0-based rank -> 6th smallest
EPS = 1e-8
WARM_MM = 24         # tiny N=128 warm matmuls (~2.5us cold) for the HAM ramp

TRACE = bool(int(os.environ.get("KNN_TRACE", "0")))
LAST_EXEC_NS = None

def _build_nc():
    import concourse.mybir as mybir
    from concourse import bacc
    from concourse.tile import TileContext

    dt = mybir.dt
    nc = bacc.Bacc(None, target_bir_lowering=False, enable_partition_id=False)

    # per-tile layout [CT][128 part][KC*CTILE contiguous] -> one DMA per tile
    ft_d = nc.dram_tensor("ft", [CT, 128, KC * CTILE], dt.float8e4, kind="ExternalInput")
    # query cols r-major per partition: the r0/r1 pair is one 2KB-line DMA
    # (what the first two row groups gate on) and r2..r7 one 6KB-line DMA —
    # wide per-partition lines roughly double the descriptor-limited DMA rate
    # vs eight 1KB-line transfers
    qt_d = nc.dram_tensor("qt", [128, RT * KC * 128], dt.float8e4, kind="ExternalInput")
    sqc_d = nc.dram_tensor("sqc", [128, N], dt.bfloat16, kind="ExternalInput")
    # raw per-row-tile top-8 T values; the density epilogue runs on the host
    out_d = nc.dram_tensor("out", [128, RT * 8], dt.float32, kind="ExternalOutput")

    DR = mybir.MatmulPerfMode.DoubleRow
    Copy = mybir.ActivationFunctionType.Copy

    with TileContext(nc) as tc:
        with (
            tc.tile_pool(name="persist", bufs=1) as persist,
            tc.tile_pool(name="ftp", bufs=3) as ftp,
            tc.tile_pool(name="psum", bufs=8, space="PSUM") as psum,
        ):
            # r-tile-major layout: each query r-tile is a fully contiguous
            # [128, KC*128] block -> its DMA is one descriptor per partition
            qt_s = persist.tile([128, RT, KC, 128], dt.float8e4)
            sqc_s = persist.tile([128, N], dt.bfloat16)
            # per r: 16 tile top-8 slots + one half-merge slot at offset 128
            cand = persist.tile([128, RT, CT * 8 + 8], dt.float32)
            top8s = persist.tile([128, RT, 8], dt.float32)
            warm = persist.tile([128, 128], dt.bfloat16)
            act_scr = persist.tile([128, 8], dt.float32)
            thr_scr = persist.tile([128, 8], dt.float8e4)

            def _qt_src(r0, r1):
                return qt_d[:, r0 * KC * 128:r1 * KC * 128].rearrange(
                    "p (r k i) -> p r k i", r=r1 - r0, k=KC)

            def _ft_src(t):
                return ft_d[t].rearrange("p (k j) -> p k j", k=KC)

            # ---- head DMA issue: need order over two HWDGE rings.  Early
            # transfers round-robin the HBM pipe, so only what the first row
            # groups gate on (qt tiles, ft0, ft1) may be in flight early;
            # sqc chunks issue behind ft2 in the loop.
            # ACT ring: qt0 alone first (the 1.3us table load right after it
            # is a deliberate issue gap that keeps the ring clear while qt0
            # drains), then qt1..qt7 at the ~0.65us/DMA issue cadence.
            nc.gpsimd.memset(warm, 0.0)
            nc.scalar.dma_start(qt_s[:, 0:1], _qt_src(0, 1))
            nc.scalar.activation(act_scr, warm[:, 0:8], Copy)
            for r in range(1, RT):
                nc.scalar.dma_start(qt_s[:, r:r + 1], _qt_src(r, r + 1))
            # SP ring: ft0's two k-halves get the ring to themselves; a tiny
            # copy gated on the second half's arrival blocks the SP queue so
            # ft1/sqc stay off the ring until ft0 has drained.
            ft_t0 = ftp.tile([128, KC, CTILE], dt.float8e4, tag="ft")
            nc.sync.dma_start(ft_t0[:, 0:KC // 2], _ft_src(0)[:, 0:KC // 2])
            nc.sync.dma_start(ft_t0[:, KC // 2:], _ft_src(0)[:, KC // 2:])
            ft_tiles = [ft_t0]
            nc.sync.dma_start(thr_scr, ft_t0[:, KC - 1, 0:8])
            ft_t1 = ftp.tile([128, KC, CTILE], dt.float8e4, tag="ft")
            nc.sync.dma_start(ft_t1, _ft_src(1))
            ft_tiles.append(ft_t1)
            nc.sync.dma_start(sqc_s[:, 0:1024], sqc_d[:, 0:1024])

            # warm-up: PE busy from engine-ready until real operands land.
            # No PSUM state is consumed downstream (t=0 groups use start=True).
            wps = [psum.tile([128, CTILE], dt.float32, tag="ps",
                             name=f"wps{b}") for b in range(3)]
            for i in range(WARM_MM):
                nc.tensor.matmul(wps[i % 3][:, 0:128], lhsT=warm, rhs=warm,
                                 start=True, stop=True)

            out_v = out_d.rearrange("p (r e) -> p r e", r=RT)
            ps_cur = [None] * RT
            for t in range(CT):
                if t == 1:
                    # remaining sqc chunks issue here so their transfers stay
                    # out of the congested head window
                    nc.sync.dma_start(sqc_s[:, 1024:4096], sqc_d[:, 1024:4096])
                if t < 2:
                    ft_t = ft_tiles[t]
                else:
                    if t == 2:
                        nc.sync.dma_start(sqc_s[:, 4096:N], sqc_d[:, 4096:N])
                    ft_t = ftp.tile([128, KC, CTILE], dt.float8e4, tag="ft")
                    nc.sync.dma_start(ft_t, _ft_src(t))
                sqc_t = sqc_s[:, t * CTILE:(t + 1) * CTILE]
                for r in range(RT):
                    if t == 0:
                        # start=True group: resets the bank's pending-zero
                        # state (left by the previous NEFF) and needs no
                        # pre-seed; DVE applies the -(sq[j]-sbar) bias after
                        ps = psum.tile([128, CTILE], dt.float32, tag="ps")
                        for k in range(0, KC, 2):
                            nc.tensor.matmul(
                                ps,
                                lhsT=qt_s[:, r, k:k + 2, :],
                                rhs=ft_t[:, k:k + 2, :],
                                start=(k == 0),
                                stop=(k == KC - 2),
                                perf_mode=DR,
                                skip_group_check=True,
                            )
                        nc.vector.tensor_sub(ps, ps, sqc_t)
                    else:
                        # bank was pre-seeded with -(sq[j]-sbar) at the end
                        # of tile t-1; fp8 matmuls accumulate 2*G on top
                        ps = ps_cur[r]
                        for k in range(0, KC, 2):
                            nc.tensor.matmul(
                                ps,
                                lhsT=qt_s[:, r, k:k + 2, :],
                                rhs=ft_t[:, k:k + 2, :],
                                start=False,
                                stop=(k == KC - 2),
                                perf_mode=DR,
                                skip_group_check=True,
                            )
                    nc.vector.max(
                        out=cand[:, r, t * 8:(t + 1) * 8],
                        in_=ps,
                    )
                    if t == CT // 2 - 1:
                        # half-merge the first 8 tiles' candidates into slot
                        # 128:136 while DVE has slack, so the t=15 final scans
                        # 72 elements instead of 128
                        nc.vector.max(out=cand[:, r, CT * 8:CT * 8 + 8],
                                      in_=cand[:, r, 0:CT * 4])
                    if t == CT - 1:
                        # final top-8 for row-tile r: second-half slots + the
                        # half-merge slot, issued as soon as r's last tile is
                        # done so the tail after the last matmul stays short
                        nc.vector.max(out=top8s[:, r, :],
                                      in_=cand[:, r, CT * 4:CT * 8 + 8])
                        if r == RT - 2:
                            # overlap most of the output DMA with r=7's tail
                            nc.sync.dma_start(out_v[:, 0:RT - 1], top8s[:, 0:RT - 1])
                # pre-allocate + seed tile t+1's PSUM banks now: each seed is
                # gated only on its bank's t-tile readers (sub/max8), so
                # ScalarE runs a full bank cycle ahead of the PE group starts
                if t + 1 < CT:
                    sqc_n = sqc_s[:, (t + 1) * CTILE:(t + 2) * CTILE]
                    nxt = []
                    for r in range(RT):
                        p = psum.tile([128, CTILE], dt.float32, tag="ps")
                        nc.scalar.activation(p, sqc_n, Copy, scale=-1.0)
                        nxt.append(p)
                    ps_cur = nxt

            nc.sync.dma_start(out_v[:, RT - 1:RT], top8s[:, RT - 1:RT])

    # run Bacc's passes (register allocation, event-semaphore wait splitting)
    # before handing off to the PJRT path, which binds without finalizing
    nc.finalize()
    return nc


def kernel(features):
    global LAST_EXEC_NS
    from concourse.bass_utils import run_bass_kernel_spmd

    f32 = np.ascontiguousarray(np.asarray(features, dtype=np.float32))
    assert f32.shape == (N, D)

    sq = np.einsum("nd,nd->n", f32, f32, dtype=np.float32)   # exact fp32 norms
    sbar = float(sq.mean())
    ftq = f32.T.astype(ml_dtypes.float8_e4m3fn)               # [D, N] fp8
    # moving operand pre-scaled by 2 (exact in fp8) so PSUM accumulates 2*G
    ft2 = (ftq.astype(np.float32) * 2.0).astype(ml_dtypes.float8_e4m3fn)
    # [D, N] -> [CT, 128, KC*CTILE]: per column tile, partition p holds all
    # KC chunks contiguously -> a single fully-contiguous DMA per tile
    ft_tiles = np.ascontiguousarray(
        ft2.reshape(KC, 128, CT, CTILE).transpose(2, 1, 0, 3).reshape(CT, 128, KC * CTILE)
    )
    sqc_rep = np.ascontiguousarray(
        np.broadcast_to((sq - sbar).astype(ml_dtypes.bfloat16), (128, N))
    )

    in_maps = []
    for c in range(NCORES):
        lo = c * ROWS
        # [128, RT*KC*128]: partition p holds all r-tiles' k-chunks r-major,
        # so qt ships as two wide-line DMAs (r0/r1, r2..r7)
        qt = np.ascontiguousarray(
            ftq[:, lo:lo + ROWS].reshape(KC, 128, RT, 128)
            .transpose(1, 2, 0, 3).reshape(128, RT * KC * 128)
        )
        in_maps.append({"ft": ft_tiles, "qt": qt, "sqc": sqc_rep})

    nc = _build_nc()
    res = run_bass_kernel_spmd(nc, in_maps, core_ids=list(range(NCORES)), trace=TRACE)
    LAST_EXEC_NS = res.exec_time_ns

    # host epilogue with exact fp32 norms: T6[p, r] holds the 6th-largest
    # 2G-sqc for global row c*1024 + r*128 + p; kth_d2 = sq[i] + sbar - T6
    dens = []
    for c in range(NCORES):
        t6 = res.results[c]["out"].reshape(128, RT, 8)[:, :, K_ORD]   # [128, RT]
        sqi = (sq[c * ROWS:(c + 1) * ROWS] + sbar).reshape(RT, 128).T
        kd = np.maximum(sqi.astype(np.float32) - t6, 0.0, dtype=np.float32)
        dens.append((1.0 / (np.sqrt(kd) + EPS)).T.reshape(-1))
    return np.concatenate(dens).astype(np.float32)[:, None]
